# revision 18
# baseline (speedup 1.0000x reference)
"""Trainium2 Bass kernel for multi-head self-attention (B=4, N=2048, C=1024, H=16).

Sharding: 8 cores = 4 batches x 2 head-groups (8 heads each). Each core:
  - computes Q^T/K^T (transposed layout) and V for its 8 heads from x[b]
  - flash-style attention: S^T tiles -> exp -> PV with a fused ones-column
    producing per-query softmax sums in the same matmul
  - normalizes O^T by 1/sum and applies its partial output projection
Host: preps per-core inputs (transpose + bf16 cast + weight column select),
adds the two partial projection outputs per batch (the tensor-parallel
reduce), and concatenates batches. No device collectives.
"""

import numpy as np
import ml_dtypes

import concourse.bass as bass
import concourse.mybir as mybir
import concourse.tile as tile
from concourse import bacc
from concourse.ap import AP
from concourse.bass_utils import run_bass_kernel_spmd

BF16 = mybir.dt.bfloat16
F32 = mybir.dt.float32
Exp = mybir.ActivationFunctionType.Exp
bf = ml_dtypes.bfloat16

B, N, C = 4, 2048, 1024
H, D = 16, 64
N_CORES = 8
HPC = H // 2  # heads per core (8)
PAIRS = HPC // 2  # head pairs per core (4)
CT = C // 128  # contraction tiles over C (8)
KT = N // 128  # key tiles (16)
RT = N // 128  # row tiles for V (16)
QC = N // 1024  # 1024-wide q chunks (2)
QT4 = N // 512  # 512-wide q chunks (4)
SCALE = 1.0 / float(np.sqrt(D))

_COMPILED = {}


def _build(with_bias: bool):
    nc = bacc.Bacc("TRN2", target_bir_lowering=False, debug=False,
                   num_devices=N_CORES)
    xt_d = nc.dram_tensor("xt", [C, N], BF16, kind="ExternalInput").ap()
    wqk_d = nc.dram_tensor("wqk", [C, 1024], BF16, kind="ExternalInput").ap()
    wv_d = nc.dram_tensor("wv", [C, 512], BF16, kind="ExternalInput").ap()
    wpr_d = nc.dram_tensor("wpr", [512, C], BF16, kind="ExternalInput").ap()
    if with_bias:
        bqk_d = nc.dram_tensor("bqk", [1, 1024], BF16, kind="ExternalInput").ap()
        bv_d = nc.dram_tensor("bv", [1, 512], BF16, kind="ExternalInput").ap()
        bpr_d = nc.dram_tensor("bpr", [1, C], BF16, kind="ExternalInput").ap()
    out_d = nc.dram_tensor("out", [N, C], F32, kind="ExternalOutput").ap()

    with tile.TileContext(nc) as tc:
        with (
            tc.tile_pool(name="persist", bufs=1) as pp,
            tc.tile_pool(name="pt", bufs=8) as pt_pool,
            tc.tile_pool(name="stage", bufs=2) as stage_pool,
            tc.tile_pool(name="bc", bufs=3) as bc_pool,
            tc.tile_pool(name="rc", bufs=3) as rc_pool,
            tc.tile_pool(name="oc", bufs=2) as oc_pool,
            tc.tile_pool(name="oa", bufs=3) as oa_pool,
            tc.tile_pool(name="ps_sc", bufs=2, space="PSUM") as ps_sc,
            tc.tile_pool(name="ps_pv", bufs=2, space="PSUM") as ps_pv,
        ):
            # x^T tiles loaded in 512-column chunks so the first V row-tiles
            # can start before the full 4MB lands
            xt_sb = [pp.tile([128, N], BF16, tag=f"xt{ct}", name=f"xt{ct}")
                     for ct in range(CT)]
            wv_sb = [pp.tile([128, 512], BF16, tag=f"wv{ct}", name=f"wv{ct}")
                     for ct in range(CT)]
            for ct in range(CT):
                nc.sync.dma_start(wv_sb[ct][:], wv_d[ct * 128:(ct + 1) * 128, :])
                nc.sync.dma_start(
                    xt_sb[ct][:, 0:512], xt_d[ct * 128:(ct + 1) * 128, 0:512])
            for q4 in range(1, QT4):
                qsl = slice(q4 * 512, (q4 + 1) * 512)
                for ct in range(CT):
                    nc.sync.dma_start(
                        xt_sb[ct][:, qsl], xt_d[ct * 128:(ct + 1) * 128, qsl])
            wqk_sb = []
            for ct in range(CT):
                t = pp.tile([128, 1024], BF16, tag=f"wqk{ct}", name=f"wqk{ct}")
                nc.sync.dma_start(t[:], wqk_d[ct * 128:(ct + 1) * 128, :])
                wqk_sb.append(t)
            wpr_sb = []
            for cp in range(PAIRS):
                t = pp.tile([128, C], BF16, tag=f"wpr{cp}", name=f"wpr{cp}")
                nc.sync.dma_start(t[:], wpr_d[cp * 128:(cp + 1) * 128, :])
                wpr_sb.append(t)
            if with_bias:
                ones = pp.tile([1, N], BF16, tag="ones")
                nc.vector.memset(ones[:], 1.0)
                bqk_sb = pp.tile([1, 1024], BF16, tag="bqk")
                nc.sync.dma_start(bqk_sb[:], bqk_d[:])
                bv_sb = pp.tile([1, 512], BF16, tag="bv")
                nc.sync.dma_start(bv_sb[:], bv_d[:])
                bpr_sb = pp.tile([1, C], BF16, tag="bpr")
                nc.sync.dma_start(bpr_sb[:], bpr_d[:])

            qt_sb = [pp.tile([128, N], BF16, tag=f"qt{p}", name=f"qt{p}")
                     for p in range(PAIRS)]
            kt_sb = [pp.tile([128, N], BF16, tag=f"kt{p}", name=f"kt{p}")
                     for p in range(PAIRS)]
            va_sb = [pp.tile([128, HPC * 65], BF16, tag=f"va{rt}", name=f"va{rt}")
                     for rt in range(RT)]
            plhsT = [pp.tile([128, N], BF16, tag=f"pl{p}", name=f"pl{p}")
                     for p in range(PAIRS)]

            # ---- V rows first: out[r, hd] = sum_c xT[c, r] * Wv[c, hd],
            # interleaved with a ones column per head (col h*65+64) that makes
            # the PV matmul also produce the softmax row-sums.
            for rt in range(RT):
                nc.vector.memset(va_sb[rt][:], 1.0)
                rsl = slice(rt * 128, (rt + 1) * 128)
                acc = ps_sc.tile([128, 1024], F32, tag="sc", name="acc_v")
                for ct in range(CT):
                    nc.tensor.matmul(acc[:, 0:512], xt_sb[ct][:, rsl],
                                     wv_sb[ct][:],
                                     start=(ct == 0), stop=(not with_bias and ct == CT - 1))
                if with_bias:
                    nc.tensor.matmul(acc[:, 0:512], ones[0:1, 0:128], bv_sb[:],
                                     start=False, stop=True)
                dst3 = va_sb[rt].rearrange("p (h d) -> p h d", d=65)[:, :, 0:64]
                src3 = acc[:, 0:512].rearrange("p (h d) -> p h d", d=64)
                nc.vector.tensor_copy(dst3, src3)

            # ---- Q^T / K^T packs for one pair: out[o, q] = sum_c W[c,o]*xT[c,q]
            def qk_chunk(p, i):
                dst, col0 = ((qt_sb, 0), (kt_sb, 512))[i // QT4]
                q4 = i % QT4
                osl = slice(col0 + p * 128, col0 + (p + 1) * 128)
                qsl = slice(q4 * 512, (q4 + 1) * 512)
                acc = ps_sc.tile([128, 1024], F32, tag="sc", name="acc_qk")
                for ct in range(CT):
                    nc.tensor.matmul(acc[:, 0:512], wqk_sb[ct][:, osl],
                                     xt_sb[ct][:, qsl], start=(ct == 0),
                                     stop=(not with_bias and ct == CT - 1))
                if with_bias:
                    nc.tensor.matmul(acc[:, 0:512], bqk_sb[0:1, osl],
                                     ones[0:1, qsl], start=False, stop=True)
                nc.vector.tensor_copy(dst[p][:, qsl], acc[:, 0:512])

            def qk_pack(p):
                for i in range(2 * QT4):
                    qk_chunk(p, i)

            # ---- attention for (pair, 1024-wide q chunk) ----
            def attention(p, qc, filler=None):
                qsl = slice(qc * 1024, (qc + 1) * 1024)
                o_ps = [ps_pv.tile([65, 1024], F32, tag="pv",
                                   name=f"o_ps{p}_{qc}_{i}") for i in range(2)]
                for kt in range(KT):
                    if filler is not None:
                        filler(kt)
                    ksl = slice(kt * 128, (kt + 1) * 128)
                    # interleave the two heads' score matmuls so the
                    # row-group-0 and row-group-64 instructions are adjacent
                    # and run concurrently in the PE array
                    scs = []
                    for hl in range(2):
                        scs.append(ps_sc.tile([128, 1024], F32, tag="sc",
                                              name="sc_att"))
                    for qh in range(2):
                        q0 = qc * 1024 + qh * 512
                        for hl in range(2):
                            pb = hl * 64
                            nc.tensor.matmul(
                                scs[hl][:, qh * 512:(qh + 1) * 512],
                                kt_sb[p][pb:pb + 64, ksl],
                                qt_sb[p][pb:pb + 64, q0:q0 + 512],
                                start=True, stop=True)
                    pts = []
                    for hl in range(2):
                        pt = pt_pool.tile([128, 1024], BF16, tag="pt", name="pt")
                        nc.scalar.activation(pt[:], scs[hl][:], Exp, scale=SCALE)
                        pts.append(pt)
                    for hl in range(2):
                        lh = 2 * p + hl
                        for qh in range(2):
                            nc.tensor.matmul(
                                o_ps[hl][:, qh * 512:(qh + 1) * 512],
                                va_sb[kt][:, lh * 65:(lh + 1) * 65],
                                pts[hl][:, qh * 512:(qh + 1) * 512],
                                start=(kt == 0), stop=(kt == KT - 1))
                # normalize O^T by 1/rowsum, store as proj lhsT (bf16).
                # First a quick PSUM->SBUF copy so the PV PSUM slot frees
                # immediately and the next segment's matmuls can start.
                for hl in range(2):
                    oa = oa_pool.tile([65, 1024], F32, tag="oa", name="oa")
                    nc.vector.tensor_copy(oa[:], o_ps[hl][:])
                    recip = rc_pool.tile([1, 1024], F32, tag="rc", name="recip")
                    nc.vector.reciprocal(recip[:], oa[64:65, :])
                    bcst = bc_pool.tile([64, 1024], F32, tag="bc", name="bcst")
                    src = AP(recip.tensor, recip.offset,
                             [recip.ap[0], [0, 64]] + list(recip.ap[1:]))
                    nc.sync.dma_start(bcst[:], src)
                    if hl == 0:
                        nc.vector.tensor_mul(plhsT[p][0:64, qsl],
                                             oa[0:64, :], bcst[:])
                    else:
                        st = stage_pool.tile([64, 1024], BF16, tag="st", name="st")
                        nc.vector.tensor_mul(st[:], oa[0:64, :], bcst[:])
                        nc.sync.dma_start(plhsT[p][64:128, qsl], st[:])

            # ---- partial output projection for one 128-row q tile ----
            def proj_tile(qt_i):
                qsl = slice(qt_i * 128, (qt_i + 1) * 128)
                oc = oc_pool.tile([128, 1024], F32, tag="oc", name="oc")
                for nch in range(2):
                    nsl = slice(nch * 512, (nch + 1) * 512)
                    acc = ps_sc.tile([128, 1024], F32, tag="sc", name="acc_pr")
                    for cp in range(PAIRS):
                        nc.tensor.matmul(acc[:, 0:512], plhsT[cp][:, qsl],
                                         wpr_sb[cp][:, nsl], start=(cp == 0),
                                         stop=(not with_bias and cp == PAIRS - 1))
                    if with_bias:
                        nc.tensor.matmul(acc[:, 0:512], ones[0:1, 0:128],
                                         bpr_sb[0:1, nsl], start=False,
                                         stop=True)
                    nc.vector.tensor_copy(oc[:, nsl], acc[:, 0:512])
                nc.sync.dma_start(out_d[qsl, :], oc[:])

            def qk_filler(p, kt):
                # spread the next pair's QK projection through this pair's
                # ACT-bound attention loop, one accumulation group at a time
                if kt % 2 == 1:
                    qk_chunk(p + 1, kt // 2)

            def proj_filler(p, kt):
                # qc0's projection tiles fill the ACT-bound gaps of qc1
                if kt in (5, 11):
                    proj_tile(2 * p + (kt > 5))

            qk_pack(0)
            for p in range(PAIRS):
                if p + 1 < PAIRS:
                    attention(p, 0, filler=lambda kt, p=p: qk_filler(p, kt))
                else:
                    attention(p, 0)
            for p in range(PAIRS):
                attention(p, 1, filler=lambda kt, p=p: proj_filler(p, kt))
            for qt_i in range(8, 16):
                proj_tile(qt_i)

    nc.compile()
    return nc


def _get_nc(with_bias=False):
    if with_bias not in _COMPILED:
        _COMPILED[with_bias] = _build(with_bias)
    return _COMPILED[with_bias]


def _prep_in_maps(x, W_qkv, b_qkv, W_proj, b_proj, with_bias):
    in_maps = []
    for c in range(N_CORES):
        b = c // 2
        g = c % 2
        hs = slice(g * 512, (g + 1) * 512)
        xt = np.ascontiguousarray(x[b].T).astype(bf)
        wq = W_qkv[:, 0:C][:, hs]
        wk = W_qkv[:, C:2 * C][:, hs]
        wv = W_qkv[:, 2 * C:3 * C][:, hs]
        wqk = np.ascontiguousarray(np.concatenate([wq, wk], axis=1)).astype(bf)
        wpr = np.ascontiguousarray(W_proj[hs, :]).astype(bf)
        m = {
            "xt": xt, "wqk": wqk, "wv": np.ascontiguousarray(wv).astype(bf),
            "wpr": wpr,
        }
        if with_bias:
            bq = b_qkv[0:C][hs]
            bk = b_qkv[C:2 * C][hs]
            bvv = b_qkv[2 * C:3 * C][hs]
            m["bqk"] = np.concatenate([bq, bk])[None, :].astype(bf)
            m["bv"] = np.ascontiguousarray(bvv[None, :]).astype(bf)
            m["bpr"] = ((b_proj if g == 0 else np.zeros_like(b_proj))
                        [None, :].astype(bf))
        in_maps.append(m)
    return in_maps


def kernel(x, W_qkv, b_qkv, W_proj, b_proj):
    x = np.asarray(x, dtype=np.float32)
    W_qkv = np.asarray(W_qkv, dtype=np.float32)
    b_qkv = np.asarray(b_qkv, dtype=np.float32)
    W_proj = np.asarray(W_proj, dtype=np.float32)
    b_proj = np.asarray(b_proj, dtype=np.float32)
    with_bias = bool(np.any(b_qkv) or np.any(b_proj))
    nc = _get_nc(with_bias)
    in_maps = _prep_in_maps(x, W_qkv, b_qkv, W_proj, b_proj, with_bias)
    res = run_bass_kernel_spmd(nc, in_maps, core_ids=list(range(N_CORES)))
    out = np.empty((B, N, C), dtype=np.float32)
    for b in range(B):
        out[b] = res.results[2 * b]["out"] + res.results[2 * b + 1]["out"]
    return out


# revision 19
# speedup vs baseline: 1.0140x; 1.0140x over previous
"""Trainium2 Bass kernel for multi-head self-attention (B=4, N=2048, C=1024, H=16).

Sharding: 8 cores = 4 batches x 2 head-groups (8 heads each). Each core:
  - computes Q^T/K^T (transposed layout) and V for its 8 heads from x[b]
  - flash-style attention: S^T tiles -> exp -> PV with a fused ones-column
    producing per-query softmax sums in the same matmul
  - normalizes O^T by 1/sum and applies its partial output projection
Host: preps per-core inputs (transpose + bf16 cast + weight column select),
adds the two partial projection outputs per batch (the tensor-parallel
reduce), and concatenates batches. No device collectives.
"""

import numpy as np
import ml_dtypes

import concourse.bass as bass
import concourse.mybir as mybir
import concourse.tile as tile
from concourse import bacc
from concourse.ap import AP
from concourse.bass_utils import run_bass_kernel_spmd

BF16 = mybir.dt.bfloat16
F32 = mybir.dt.float32
Exp = mybir.ActivationFunctionType.Exp
bf = ml_dtypes.bfloat16

B, N, C = 4, 2048, 1024
H, D = 16, 64
N_CORES = 8
HPC = H // 2  # heads per core (8)
PAIRS = HPC // 2  # head pairs per core (4)
CT = C // 128  # contraction tiles over C (8)
KT = N // 128  # key tiles (16)
RT = N // 128  # row tiles for V (16)
QC = N // 1024  # 1024-wide q chunks (2)
QT4 = N // 512  # 512-wide q chunks (4)
SCALE = 1.0 / float(np.sqrt(D))

_COMPILED = {}


def _build(with_bias: bool):
    nc = bacc.Bacc("TRN2", target_bir_lowering=False, debug=False,
                   num_devices=N_CORES)
    xt_d = nc.dram_tensor("xt", [C, N], BF16, kind="ExternalInput").ap()
    wqk_d = nc.dram_tensor("wqk", [C, 1024], BF16, kind="ExternalInput").ap()
    wv_d = nc.dram_tensor("wv", [C, 512], BF16, kind="ExternalInput").ap()
    wpr_d = nc.dram_tensor("wpr", [512, C], BF16, kind="ExternalInput").ap()
    if with_bias:
        bqk_d = nc.dram_tensor("bqk", [1, 1024], BF16, kind="ExternalInput").ap()
        bv_d = nc.dram_tensor("bv", [1, 512], BF16, kind="ExternalInput").ap()
        bpr_d = nc.dram_tensor("bpr", [1, C], BF16, kind="ExternalInput").ap()
    out_d = nc.dram_tensor("out", [N, C], F32, kind="ExternalOutput").ap()

    with tile.TileContext(nc) as tc:
        with (
            tc.tile_pool(name="persist", bufs=1) as pp,
            tc.tile_pool(name="pt", bufs=8) as pt_pool,
            tc.tile_pool(name="stage", bufs=2) as stage_pool,
            tc.tile_pool(name="bc", bufs=3) as bc_pool,
            tc.tile_pool(name="rc", bufs=3) as rc_pool,
            tc.tile_pool(name="oc", bufs=2) as oc_pool,
            tc.tile_pool(name="oa", bufs=3) as oa_pool,
            tc.tile_pool(name="ps_sc", bufs=2, space="PSUM") as ps_sc,
            tc.tile_pool(name="ps_pv", bufs=2, space="PSUM") as ps_pv,
        ):
            # x^T tiles loaded in 512-column chunks so the first V row-tiles
            # can start before the full 4MB lands
            xt_sb = [pp.tile([128, N], BF16, tag=f"xt{ct}", name=f"xt{ct}")
                     for ct in range(CT)]
            wv_sb = [pp.tile([128, 512], BF16, tag=f"wv{ct}", name=f"wv{ct}")
                     for ct in range(CT)]
            for ct in range(CT):
                nc.sync.dma_start(wv_sb[ct][:], wv_d[ct * 128:(ct + 1) * 128, :])
                nc.sync.dma_start(
                    xt_sb[ct][:, 0:512], xt_d[ct * 128:(ct + 1) * 128, 0:512])
            for q4 in range(1, QT4):
                qsl = slice(q4 * 512, (q4 + 1) * 512)
                for ct in range(CT):
                    nc.sync.dma_start(
                        xt_sb[ct][:, qsl], xt_d[ct * 128:(ct + 1) * 128, qsl])
            wqk_sb = []
            for ct in range(CT):
                t = pp.tile([128, 1024], BF16, tag=f"wqk{ct}", name=f"wqk{ct}")
                nc.sync.dma_start(t[:], wqk_d[ct * 128:(ct + 1) * 128, :])
                wqk_sb.append(t)
            wpr_sb = []
            for cp in range(PAIRS):
                t = pp.tile([128, C], BF16, tag=f"wpr{cp}", name=f"wpr{cp}")
                nc.sync.dma_start(t[:], wpr_d[cp * 128:(cp + 1) * 128, :])
                wpr_sb.append(t)
            if with_bias:
                ones = pp.tile([1, N], BF16, tag="ones")
                nc.vector.memset(ones[:], 1.0)
                bqk_sb = pp.tile([1, 1024], BF16, tag="bqk")
                nc.sync.dma_start(bqk_sb[:], bqk_d[:])
                bv_sb = pp.tile([1, 512], BF16, tag="bv")
                nc.sync.dma_start(bv_sb[:], bv_d[:])
                bpr_sb = pp.tile([1, C], BF16, tag="bpr")
                nc.sync.dma_start(bpr_sb[:], bpr_d[:])

            qt_sb = [pp.tile([128, N], BF16, tag=f"qt{p}", name=f"qt{p}")
                     for p in range(PAIRS)]
            kt_sb = [pp.tile([128, N], BF16, tag=f"kt{p}", name=f"kt{p}")
                     for p in range(PAIRS)]
            va_sb = [pp.tile([128, HPC * 65], BF16, tag=f"va{rt}", name=f"va{rt}")
                     for rt in range(RT)]
            plhsT = [pp.tile([128, N], BF16, tag=f"pl{p}", name=f"pl{p}")
                     for p in range(PAIRS)]

            # ---- V rows first: out[r, hd] = sum_c xT[c, r] * Wv[c, hd],
            # interleaved with a ones column per head (col h*65+64) that makes
            # the PV matmul also produce the softmax row-sums.
            for rt in range(RT):
                nc.vector.memset(va_sb[rt][:], 1.0)
                rsl = slice(rt * 128, (rt + 1) * 128)
                acc = ps_sc.tile([128, 1024], F32, tag="sc", name="acc_v")
                for ct in range(CT):
                    nc.tensor.matmul(acc[:, 0:512], xt_sb[ct][:, rsl],
                                     wv_sb[ct][:],
                                     start=(ct == 0), stop=(not with_bias and ct == CT - 1))
                if with_bias:
                    nc.tensor.matmul(acc[:, 0:512], ones[0:1, 0:128], bv_sb[:],
                                     start=False, stop=True)
                dst3 = va_sb[rt].rearrange("p (h d) -> p h d", d=65)[:, :, 0:64]
                src3 = acc[:, 0:512].rearrange("p (h d) -> p h d", d=64)
                nc.vector.tensor_copy(dst3, src3)

            # ---- Q^T / K^T packs for one pair: out[o, q] = sum_c W[c,o]*xT[c,q]
            def qk_chunk(p, i):
                dst, col0 = ((qt_sb, 0), (kt_sb, 512))[i // QT4]
                q4 = i % QT4
                osl = slice(col0 + p * 128, col0 + (p + 1) * 128)
                qsl = slice(q4 * 512, (q4 + 1) * 512)
                acc = ps_sc.tile([128, 1024], F32, tag="sc", name="acc_qk")
                for ct in range(CT):
                    nc.tensor.matmul(acc[:, 0:512], wqk_sb[ct][:, osl],
                                     xt_sb[ct][:, qsl], start=(ct == 0),
                                     stop=(not with_bias and ct == CT - 1))
                if with_bias:
                    nc.tensor.matmul(acc[:, 0:512], bqk_sb[0:1, osl],
                                     ones[0:1, qsl], start=False, stop=True)
                nc.vector.tensor_copy(dst[p][:, qsl], acc[:, 0:512])

            def qk_pack(p):
                for i in range(2 * QT4):
                    qk_chunk(p, i)

            # ---- attention for (pair, 1024-wide q chunk) ----
            def attention(p, qc, filler=None):
                qsl = slice(qc * 1024, (qc + 1) * 1024)
                o_ps = [ps_pv.tile([65, 1024], F32, tag="pv",
                                   name=f"o_ps{p}_{qc}_{i}") for i in range(2)]
                for kt in range(KT):
                    if filler is not None:
                        filler(kt)
                    ksl = slice(kt * 128, (kt + 1) * 128)
                    # interleave the two heads' score matmuls so the
                    # row-group-0 and row-group-64 instructions are adjacent
                    # and run concurrently in the PE array
                    scs = []
                    for hl in range(2):
                        scs.append(ps_sc.tile([128, 1024], F32, tag="sc",
                                              name="sc_att"))
                    for qh in range(2):
                        q0 = qc * 1024 + qh * 512
                        for hl in range(2):
                            pb = hl * 64
                            nc.tensor.matmul(
                                scs[hl][:, qh * 512:(qh + 1) * 512],
                                kt_sb[p][pb:pb + 64, ksl],
                                qt_sb[p][pb:pb + 64, q0:q0 + 512],
                                start=True, stop=True)
                    pts = []
                    for hl in range(2):
                        pt = pt_pool.tile([128, 1024], BF16, tag="pt", name="pt")
                        nc.scalar.activation(pt[:], scs[hl][:], Exp, scale=SCALE)
                        pts.append(pt)
                    for hl in range(2):
                        lh = 2 * p + hl
                        for qh in range(2):
                            nc.tensor.matmul(
                                o_ps[hl][:, qh * 512:(qh + 1) * 512],
                                va_sb[kt][:, lh * 65:(lh + 1) * 65],
                                pts[hl][:, qh * 512:(qh + 1) * 512],
                                start=(kt == 0), stop=(kt == KT - 1))
                # normalize O^T by 1/rowsum, store as proj lhsT (bf16).
                # First a quick PSUM->SBUF copy so the PV PSUM slot frees
                # immediately and the next segment's matmuls can start.
                for hl in range(2):
                    oa = oa_pool.tile([65, 1024], F32, tag="oa", name="oa")
                    nc.vector.tensor_copy(oa[:], o_ps[hl][:])
                    recip = rc_pool.tile([1, 1024], F32, tag="rc", name="recip")
                    nc.vector.reciprocal(recip[:], oa[64:65, :])
                    bcst = bc_pool.tile([64, 1024], F32, tag="bc", name="bcst")
                    src = AP(recip.tensor, recip.offset,
                             [recip.ap[0], [0, 64]] + list(recip.ap[1:]))
                    nc.sync.dma_start(bcst[:], src)
                    if hl == 0:
                        nc.vector.tensor_mul(plhsT[p][0:64, qsl],
                                             oa[0:64, :], bcst[:])
                    else:
                        st = stage_pool.tile([64, 1024], BF16, tag="st", name="st")
                        nc.vector.tensor_mul(st[:], oa[0:64, :], bcst[:])
                        nc.sync.dma_start(plhsT[p][64:128, qsl], st[:])

            # ---- partial output projection for one 128-row q tile ----
            def proj_tile(qt_i):
                qsl = slice(qt_i * 128, (qt_i + 1) * 128)
                oc = oc_pool.tile([128, 1024], F32, tag="oc", name="oc")
                for nch in range(2):
                    nsl = slice(nch * 512, (nch + 1) * 512)
                    acc = ps_sc.tile([128, 1024], F32, tag="sc", name="acc_pr")
                    for cp in range(PAIRS):
                        nc.tensor.matmul(acc[:, 0:512], plhsT[cp][:, qsl],
                                         wpr_sb[cp][:, nsl], start=(cp == 0),
                                         stop=(not with_bias and cp == PAIRS - 1))
                    if with_bias:
                        nc.tensor.matmul(acc[:, 0:512], ones[0:1, 0:128],
                                         bpr_sb[0:1, nsl], start=False,
                                         stop=True)
                    nc.vector.tensor_copy(oc[:, nsl], acc[:, 0:512])
                nc.sync.dma_start(out_d[qsl, :], oc[:])

            def proj_filler(p, kt):
                # qc0's projection tiles fill the ACT-bound gaps of qc1
                if kt in (5, 11):
                    proj_tile(2 * p + (kt > 5))

            qk_pack(0)
            for p in range(PAIRS):
                if p + 1 < PAIRS:
                    qk_pack(p + 1)  # overlaps with attention of pair p
                attention(p, 0)
            for p in range(PAIRS):
                attention(p, 1, filler=lambda kt, p=p: proj_filler(p, kt))
            for qt_i in range(8, 16):
                proj_tile(qt_i)

    nc.compile()
    return nc


def _get_nc(with_bias=False):
    if with_bias not in _COMPILED:
        _COMPILED[with_bias] = _build(with_bias)
    return _COMPILED[with_bias]


def _prep_in_maps(x, W_qkv, b_qkv, W_proj, b_proj, with_bias):
    in_maps = []
    for c in range(N_CORES):
        b = c // 2
        g = c % 2
        hs = slice(g * 512, (g + 1) * 512)
        xt = np.ascontiguousarray(x[b].T).astype(bf)
        wq = W_qkv[:, 0:C][:, hs]
        wk = W_qkv[:, C:2 * C][:, hs]
        wv = W_qkv[:, 2 * C:3 * C][:, hs]
        wqk = np.ascontiguousarray(np.concatenate([wq, wk], axis=1)).astype(bf)
        wpr = np.ascontiguousarray(W_proj[hs, :]).astype(bf)
        m = {
            "xt": xt, "wqk": wqk, "wv": np.ascontiguousarray(wv).astype(bf),
            "wpr": wpr,
        }
        if with_bias:
            bq = b_qkv[0:C][hs]
            bk = b_qkv[C:2 * C][hs]
            bvv = b_qkv[2 * C:3 * C][hs]
            m["bqk"] = np.concatenate([bq, bk])[None, :].astype(bf)
            m["bv"] = np.ascontiguousarray(bvv[None, :]).astype(bf)
            m["bpr"] = ((b_proj if g == 0 else np.zeros_like(b_proj))
                        [None, :].astype(bf))
        in_maps.append(m)
    return in_maps


def kernel(x, W_qkv, b_qkv, W_proj, b_proj):
    x = np.asarray(x, dtype=np.float32)
    W_qkv = np.asarray(W_qkv, dtype=np.float32)
    b_qkv = np.asarray(b_qkv, dtype=np.float32)
    W_proj = np.asarray(W_proj, dtype=np.float32)
    b_proj = np.asarray(b_proj, dtype=np.float32)
    with_bias = bool(np.any(b_qkv) or np.any(b_proj))
    nc = _get_nc(with_bias)
    in_maps = _prep_in_maps(x, W_qkv, b_qkv, W_proj, b_proj, with_bias)
    res = run_bass_kernel_spmd(nc, in_maps, core_ids=list(range(N_CORES)))
    out = np.empty((B, N, C), dtype=np.float32)
    for b in range(B):
        out[b] = res.results[2 * b]["out"] + res.results[2 * b + 1]["out"]
    return out
